# revision 40
# baseline (speedup 1.0000x reference)
"""Trainium2 Bass kernel for BinaryHead: logits = (l2norm(fea) @ W.T + b) * 16.

Sharding: data-parallel over the batch dim across 8 NeuronCores (2048 rows
each).  The host stages each core's shard TRANSPOSED ([emb, batch]) as bf16 so
the embedding/contraction dim lands on SBUF partitions, and batch-QUARTER
major ([quarter, half, 128, 8panels, 512] contiguous) so every DMA transfer
is a 1 MiB block with 8 KiB per-partition runs.

v4: column-tiled PE + chunk-major streaming + fp8 DoubleRow sumsq.  The z
matmuls have a 4-wide stationary (4 classes), so a plain matmul uses 4/128 of
the PE array and the kernel is PE-bound.  Fix 1: panel p runs on array
column-tile t=p%4 via tile_position=(0,32t) -- four matmuls stream
concurrently through disjoint column groups of the array.  Fix 2: the batch
streams in four 512-col quarters (two 1 MiB transfers each), so PE/DVE/ACT
work arrives every ~3us and each quarter's accumulation finishes while the
next quarter streams -- the normalization epilogue hides under the stream
instead of serializing at the kernel tail.  The PE runs at the HAM
half-clock (K=4/8) in this duty-cycle regime, so the sumsq matmuls lag a full
half-quarter behind their squares to keep the in-order PE queue from ever
stalling on DVE/ACT latency.  (DoubleRow + column tiling hard-locks the PE --
NRT_EXEC_UNIT_UNRECOVERABLE -- so sumsq contracts per-panel in bf16.)

Per (quarter, half):  z_t += Wt.T @ x  (col tile t),  ss_t += 1.T @ x^2
(bf16, col tile t) with squares split across DVE/ACT.  Per-quarter
epilogue: evacuate the two accumulator banks to SBUF (f32r/bf16), reduce the
4 tiles' partials with tiny selection-matrix matmuls (Rz at col tile 0, Rs at
col tile 1), rsqrt via exp(-0.5*ln(ss)+ln(S)) on ACT, class-broadcast via a
k=1 matmul (row tile 32), then DVE mul + bias add and DMA out on the scalar
ring.
"""

from contextlib import ExitStack

import numpy as np

NUM_CLASS = 4
EMB = 2048
BATCH = 16384
N_CORES = 8
ROWS = BATCH // N_CORES  # 2048 rows per core
S = 16.0

N_ETILES = EMB // 128  # 16 e-panels per core
N_H = 2  # halves (8-panel blocks) per quarter
N_Q = 4  # batch quarters
CW = 512  # quarter width (one psum bank)

# square-engine map per (quarter, half): ACT/GPSIMD cells; rest on DVE.
# Quarter 0 avoids ACT so the activation-table preload runs first; quarter 3
# avoids ACT/GPSIMD so the tail epilogue chain owns them.  GPSIMD squares are
# slow (~3.6us) but their sumsq matmuls lag a full half-quarter, so the
# in-order PE queue never waits on them.
ACT_SQ = {(1, 0), (1, 1), (2, 0)}
GPS_SQ = set()

DTYPE_CFG = "bf16"  # informational (test harness prints it)

_CACHE = {}


def _build_nc():
    import concourse.bacc as bacc
    import concourse.mybir as mybir
    import concourse.tile as tile
    from concourse.hw_specs import get_activation_tables

    f32 = mybir.dt.float32
    f32r = mybir.dt.float32r
    bf16 = mybir.dt.bfloat16
    fp8 = mybir.dt.float8e4

    nc = bacc.Bacc(
        "TRN2",
        target_bir_lowering=False,
        debug=False,
        enable_asserts=False,
        num_devices=N_CORES,
    )

    feaT = nc.dram_tensor(
        "feaT", [N_Q, N_H, 128, 8, CW], bf16, kind="ExternalInput"
    ).ap()
    wt = nc.dram_tensor(
        "wt", [128, N_ETILES * NUM_CLASS], bf16, kind="ExternalInput"
    ).ap()
    onesv = nc.dram_tensor("onesv", [128, 1], bf16, kind="ExternalInput").ap()
    selz = nc.dram_tensor("selz", [128, NUM_CLASS], f32r, kind="ExternalInput").ap()
    # bf16: the f32r matmul path cannot target a non-zero column tile
    # (s3d3_mm_valid_dst_partition), and the Rs reduce writes col tile 1
    selss = nc.dram_tensor("selss", [128, 1], bf16, kind="ExternalInput").ap()
    sones = nc.dram_tensor("sones", [1, NUM_CLASS], f32r, kind="ExternalInput").ap()
    sbias = nc.dram_tensor("sbias", [NUM_CLASS, 1], f32, kind="ExternalInput").ap()
    outT = nc.dram_tensor("outT", [NUM_CLASS, ROWS], f32, kind="ExternalOutput").ap()

    with tile.TileContext(nc) as tc, ExitStack() as ctx:
        pconst = ctx.enter_context(tc.tile_pool(name="pconst", bufs=1))
        pdata = ctx.enter_context(tc.tile_pool(name="pdata", bufs=8))
        psq = ctx.enter_context(tc.tile_pool(name="psq", bufs=5))
        pep = ctx.enter_context(tc.tile_pool(name="pep", bufs=1))
        pev = ctx.enter_context(tc.tile_pool(name="pev", bufs=2))
        pz = ctx.enter_context(tc.tile_pool(name="pz", bufs=2, space="PSUM"))
        ps = ctx.enter_context(tc.tile_pool(name="ps", bufs=2, space="PSUM"))
        pr = ctx.enter_context(tc.tile_pool(name="pr", bufs=4, space="PSUM"))

        # all consts ride SWDGE so the sync HWDGE ring starts the input
        # stream immediately (the stream is the conveyor; the warmup can wait)
        wt_s = pconst.tile([128, N_ETILES * NUM_CLASS], bf16)
        nc.gpsimd.dma_start(out=wt_s, in_=wt)
        ones_s = pconst.tile([128, 1], bf16)
        nc.gpsimd.dma_start(out=ones_s, in_=onesv)
        selz_s = pconst.tile([128, NUM_CLASS], f32r)
        nc.gpsimd.dma_start(out=selz_s, in_=selz)
        selss_s = pconst.tile([128, 1], bf16)
        nc.gpsimd.dma_start(out=selss_s, in_=selss)
        # sones placed on partition 32 (same partition as the reduced sumsq)
        sones_s = pconst.tile([128, NUM_CLASS], f32r)
        nc.gpsimd.dma_start(out=sones_s[32:33, :], in_=sones)
        sbias_s = pconst.tile([NUM_CLASS, 1], f32)
        nc.gpsimd.dma_start(out=sbias_s, in_=sbias)
        zero128_s = pconst.tile([128, 1], f32)
        nc.vector.memset(zero128_s, 0.0)
        # rsqrt via exp(-0.5*ln(ss) + ln(S)): folds the *S scale in for free
        lnS_s = pconst.tile([128, 1], f32)
        nc.vector.memset(lnS_s, float(np.log(S)))

        lnss_s = pep.tile([128, ROWS], f32)
        rnorm_s = pep.tile([128, ROWS], f32r)
        zr_s = pep.tile([NUM_CLASS, ROWS], f32)
        out_s = pep.tile([NUM_CLASS, ROWS], f32)

        # per-quarter accumulators, one psum bank each: tile t's z partial at
        # partitions 32t..32t+3, its sumsq partial at partition 32t
        zt_ps = [None] * N_Q
        ss_ps = [None] * N_Q
        xts = [[None] * N_H for _ in range(N_Q)]
        x2s = [[None] * N_H for _ in range(N_Q)]

        def issue_dma(j):
            # all inputs on the SP HWDGE ring (the ACT ring stalls the ACT
            # queue); every xt tile is pool-resident so no dma_start ever
            # waits on buffer reuse.  Mixed granularity: quarter 0 and the
            # final half land in 2-panel pieces (early start / tail chasing),
            # middle quarters in single 2 MiB transfers (peak HBM rate).
            for h in range(N_H):
                xts[j][h] = pdata.tile([128, 8, CW], bf16, tag="xt", name=f"xt{j}{h}")
            if j == 0:
                for h in range(N_H):
                    for i in range(0, 8, 2):
                        nc.sync.dma_start(
                            out=xts[j][h][:, i : i + 2, :],
                            in_=feaT[j, h, :, i : i + 2, :],
                        )
            else:
                for h in range(N_H):
                    nc.sync.dma_start(out=xts[j][h], in_=feaT[j, h])

        def square(j, h):
            x2s[j][h] = psq.tile([128, 8, CW], bf16, tag="x2", name=f"x2{j}{h}")
            if (j, h) in ACT_SQ:
                nc.scalar.activation(
                    out=x2s[j][h],
                    in_=xts[j][h],
                    func=mybir.ActivationFunctionType.Square,
                    bias=zero128_s,
                    scale=1.0,
                )
            elif (j, h) in GPS_SQ:
                nc.gpsimd.tensor_mul(x2s[j][h], xts[j][h], xts[j][h])
            elif j == N_Q - 1 and h == N_H - 1:
                # the very last half: fine-grained pair squares so the tail
                # sumsq matmuls chase pairs instead of one 4096-wide op
                for i in range(0, 8, 2):
                    nc.vector.tensor_mul(
                        x2s[j][h][:, i : i + 2, :],
                        xts[j][h][:, i : i + 2, :],
                        xts[j][h][:, i : i + 2, :],
                    )
            else:
                nc.vector.tensor_mul(x2s[j][h], xts[j][h], xts[j][h])

        def z_mms(j, h, start, stop):
            for i in range(8):
                p = 8 * h + i
                t = p % 4
                nc.tensor.matmul(
                    zt_ps[j][32 * t : 32 * t + 4, :],
                    wt_s[:, p * NUM_CLASS : (p + 1) * NUM_CLASS],
                    xts[j][h][:, i, :],
                    start=start and i < 4,
                    stop=stop and i >= 4,
                    tile_position=(0, 32 * t),
                )

        def ss_mms(j, h, start, stop):
            for i in range(8):
                t = (8 * h + i) % 4
                nc.tensor.matmul(
                    ss_ps[j][32 * t : 32 * t + 1, :],
                    ones_s,
                    x2s[j][h][:, i, :],
                    start=start and i < 4,
                    stop=stop and i >= 4,
                    tile_position=(0, 32 * t),
                )

        def epi_part1(j):
            # evacuate the two accumulator banks, reduce, ln
            bsl = slice(j * CW, (j + 1) * CW)
            zsb = pev.tile([128, CW], f32r, tag="zsb", name=f"zsb{j}")
            nc.vector.tensor_copy(zsb, zt_ps[j])
            ssb = pev.tile([128, CW], bf16, tag="ssb", name=f"ssb{j}")
            nc.scalar.copy(ssb, ss_ps[j])
            ra = pr.tile([128, CW], f32, tag="rx", name=f"ra{j}")
            nc.tensor.matmul(
                ra[0:NUM_CLASS, :],
                selz_s,
                zsb,
                start=True,
                stop=True,
                tile_position=(0, 0),
            )
            nc.tensor.matmul(
                ra[32:33, :],
                selss_s,
                ssb,
                start=True,
                stop=True,
                tile_position=(0, 32),
            )
            nc.scalar.activation(
                out=lnss_s[32:33, bsl],
                in_=ra[32:33, :],
                func=mybir.ActivationFunctionType.Ln,
                bias=zero128_s[32:33],
                scale=1.0,
            )
            nc.scalar.activation(
                out=rnorm_s[32:33, bsl],
                in_=lnss_s[32:33, bsl],
                func=mybir.ActivationFunctionType.Exp,
                bias=lnS_s[32:33],
                scale=-0.5,
            )
            return ra

        def epi_part2(j, ra):
            # rnorm broadcast (k=1 matmul on row tile 32), scale, bias, out
            bsl = slice(j * CW, (j + 1) * CW)
            rb = pr.tile([128, CW], f32, tag="rx", name=f"rb{j}")
            nc.tensor.matmul(
                rb[0:NUM_CLASS, :],
                sones_s[32:33, :],
                rnorm_s[32:33, bsl],
                start=True,
                stop=True,
                tile_position=(32, 0),
            )
            # DVE can read only one PSUM operand per op: stage rnb in SBUF
            rnbs = pev.tile([NUM_CLASS, CW], f32, tag="rnbs", name=f"rnbs{j}")
            nc.vector.tensor_copy(rnbs, rb[0:NUM_CLASS, :])
            nc.vector.tensor_mul(zr_s[:, bsl], ra[0:NUM_CLASS, :], rnbs)
            nc.vector.tensor_scalar_add(
                out_s[:, bsl], in0=zr_s[:, bsl], scalar1=sbias_s
            )
            nc.sync.dma_start(out=outT[:, bsl], in_=out_s[:, bsl])

        issue_dma(0)
        # activation-table preload (Square+Ln+Exp in one set) while the ACT
        # queue is otherwise empty; quarter 0's squares avoid ACT
        nlx_id = list(get_activation_tables(nc.m.arch)).index(
            "natural_log_exp_and_others"
        )
        nc.scalar.add_instruction(
            mybir.InstLoadActFuncSet(name=f"I-{nc.next_id()}", act_func_set_id=nlx_id)
        )

        ras = [None] * N_Q
        for j in range(N_Q):
            zt_ps[j] = pz.tile([128, CW], f32, tag="zt", name=f"zt{j}")
            ss_ps[j] = ps.tile([128, CW], f32, tag="ss", name=f"ss{j}")
            if j == 0:
                # pre-warm the PE while the first transfer is in flight: the
                # first real z matmul's start=True resets the garbage
                for w in range(24):
                    t = w % 4
                    nc.tensor.matmul(
                        zt_ps[0][32 * t : 32 * t + 4, 0:64],
                        wt_s[:, 0:NUM_CLASS],
                        wt_s[:, 0:64],
                        start=True,
                        stop=True,
                        tile_position=(0, 32 * t),
                    )
            if j + 1 < N_Q:
                issue_dma(j + 1)
            for h in range(N_H):
                square(j, h)
                z_mms(j, h, start=(h == 0), stop=(h == N_H - 1))
                if h == 0 and j > 0:
                    # previous quarter's second-half sumsq: its squares are a
                    # full quarter old, so the in-order PE queue never stalls
                    ss_mms(j - 1, N_H - 1, start=False, stop=True)
                    if j > 1:
                        # part2's broadcast matmul waits on the Ln/Exp chain;
                        # emitted a further half-slot later so it never sits
                        # in the PE queue ahead of stream/sumsq matmuls
                        epi_part2(j - 2, ras[j - 2])
                if h == 1:
                    ss_mms(j, 0, start=True, stop=False)
                    if j > 0:
                        ras[j - 1] = epi_part1(j - 1)
        ss_mms(N_Q - 1, N_H - 1, start=False, stop=True)
        epi_part2(N_Q - 2, ras[N_Q - 2])
        ras[N_Q - 1] = epi_part1(N_Q - 1)
        epi_part2(N_Q - 1, ras[N_Q - 1])

    nc.compile()
    return nc


def _get_nc():
    if "nc" not in _CACHE:
        _CACHE["nc"] = _build_nc()
    return _CACHE["nc"]


def _stage_inputs(fea, W, b):
    import ml_dtypes

    fea = np.asarray(fea, dtype=np.float32)
    W = np.asarray(W, dtype=np.float32)
    b = np.asarray(b, dtype=np.float32)

    # wt[p, 4t+c] = W[c, 128t+p]
    wt = np.ascontiguousarray(
        W.reshape(NUM_CLASS, N_ETILES, 128).transpose(2, 1, 0).reshape(128, -1)
    ).astype(ml_dtypes.bfloat16)
    onesv = np.ones((128, 1), dtype=ml_dtypes.bfloat16)
    selz = np.zeros((128, NUM_CLASS), dtype=np.float32)
    selss = np.zeros((128, 1), dtype=ml_dtypes.bfloat16)
    for t in range(4):
        for c in range(NUM_CLASS):
            selz[32 * t + c, c] = 1.0
        selss[32 * t, 0] = 1.0
    # the *S scale is folded into the exp(-0.5*ln(ss)+ln(S)) rsqrt, so the
    # class-broadcast matmul uses plain ones
    sones = np.ones((1, NUM_CLASS), dtype=np.float32)
    sbias = (S * b).reshape(NUM_CLASS, 1).astype(np.float32)

    in_maps = []
    for i in range(N_CORES):
        shard = fea[i * ROWS : (i + 1) * ROWS, :]
        feaT = np.ascontiguousarray(shard.T)  # [EMB, ROWS]
        # [quarter j, half h, partition p, panel a, col b]
        fea5 = np.ascontiguousarray(
            feaT.reshape(N_H, 8, 128, N_Q, CW).transpose(3, 0, 2, 1, 4)
        ).astype(ml_dtypes.bfloat16)
        in_maps.append(
            {
                "feaT": fea5,
                "wt": wt,
                "onesv": onesv,
                "selz": selz,
                "selss": selss,
                "sones": sones,
                "sbias": sbias,
            }
        )
    return in_maps


def run(fea, W, b, trace=False):
    from concourse.bass_utils import run_bass_kernel_spmd

    nc = _get_nc()
    in_maps = _stage_inputs(fea, W, b)
    res = run_bass_kernel_spmd(nc, in_maps, core_ids=list(range(N_CORES)), trace=trace)
    out = np.empty((BATCH, NUM_CLASS), dtype=np.float32)
    for i in range(N_CORES):
        out[i * ROWS : (i + 1) * ROWS, :] = res.results[i]["outT"].T
    return out, res


def kernel(fea, W, b):
    out, _ = run(fea, W, b, trace=False)
    return out


# revision 41
# speedup vs baseline: 1.0082x; 1.0082x over previous
"""Trainium2 Bass kernel for BinaryHead: logits = (l2norm(fea) @ W.T + b) * 16.

Sharding: data-parallel over the batch dim across 8 NeuronCores (2048 rows
each).  The host stages each core's shard TRANSPOSED ([emb, batch]) as bf16 so
the embedding/contraction dim lands on SBUF partitions, and batch-QUARTER
major ([quarter, half, 128, 8panels, 512] contiguous) so every DMA transfer
is a 1 MiB block with 8 KiB per-partition runs.

Column-tiled PE + chunk-major streaming.  The z
matmuls have a 4-wide stationary (4 classes), so a plain matmul uses 4/128 of
the PE array and the kernel is PE-bound.  Fix 1: panel p runs on array
column-tile t=p%4 via tile_position=(0,32t) -- four matmuls stream
concurrently through disjoint column groups of the array.  Fix 2: the batch
streams in four 512-col quarters (two 1 MiB transfers each), so PE/DVE/ACT
work arrives every ~3us and each quarter's accumulation finishes while the
next quarter streams -- the normalization epilogue hides under the stream
instead of serializing at the kernel tail.  The PE runs at the HAM
half-clock (K=4/8) in this duty-cycle regime, so the sumsq matmuls lag a full
half-quarter behind their squares to keep the in-order PE queue from ever
stalling on DVE/ACT latency.  (DoubleRow + column tiling hard-locks the PE --
NRT_EXEC_UNIT_UNRECOVERABLE -- so sumsq contracts per-panel in bf16.)

Per (quarter, half):  z_t += Wt.T @ x  (col tile t),  ss_t += 1.T @ x^2
(bf16, col tile t) with squares split across DVE/ACT.  Per-quarter
epilogue: evacuate the two accumulator banks to SBUF (f32r/bf16), reduce the
4 tiles' partials with tiny selection-matrix matmuls (Rz at col tile 0, Rs at
col tile 1), rsqrt via exp(-0.5*ln(ss)+ln(S)) on ACT, class-broadcast via a
k=1 matmul (row tile 32), then DVE mul + bias add and DMA out on the scalar
ring.
"""

from contextlib import ExitStack

import numpy as np

NUM_CLASS = 4
EMB = 2048
BATCH = 16384
N_CORES = 8
ROWS = BATCH // N_CORES  # 2048 rows per core
S = 16.0

N_ETILES = EMB // 128  # 16 e-panels per core
N_H = 2  # halves (8-panel blocks) per quarter
N_Q = 4  # batch quarters
CW = 512  # quarter width (one psum bank)

# square-engine map per (quarter, half): ACT/GPSIMD cells; rest on DVE.
# Quarter 0 avoids ACT so the activation-table preload runs first; quarter 3
# avoids ACT/GPSIMD so the tail epilogue chain owns them.  GPSIMD squares are
# slow (~3.6us) but their sumsq matmuls lag a full half-quarter, so the
# in-order PE queue never waits on them.
ACT_SQ = {(1, 0), (1, 1), (2, 0)}
GPS_SQ = set()

DTYPE_CFG = "bf16"  # informational (test harness prints it)

_CACHE = {}


def _build_nc():
    import concourse.bacc as bacc
    import concourse.mybir as mybir
    import concourse.tile as tile
    from concourse.hw_specs import get_activation_tables

    f32 = mybir.dt.float32
    f32r = mybir.dt.float32r
    bf16 = mybir.dt.bfloat16
    fp8 = mybir.dt.float8e4

    nc = bacc.Bacc(
        "TRN2",
        target_bir_lowering=False,
        debug=False,
        enable_asserts=False,
        num_devices=N_CORES,
    )

    feaT = nc.dram_tensor(
        "feaT", [N_Q, N_H, 128, 8, CW], bf16, kind="ExternalInput"
    ).ap()
    wt = nc.dram_tensor(
        "wt", [128, N_ETILES * NUM_CLASS], bf16, kind="ExternalInput"
    ).ap()
    onesv = nc.dram_tensor("onesv", [128, 1], bf16, kind="ExternalInput").ap()
    selz = nc.dram_tensor("selz", [128, NUM_CLASS], f32r, kind="ExternalInput").ap()
    # bf16: the f32r matmul path cannot target a non-zero column tile
    # (s3d3_mm_valid_dst_partition), and the Rs reduce writes col tile 1
    selss = nc.dram_tensor("selss", [128, 1], bf16, kind="ExternalInput").ap()
    sones = nc.dram_tensor("sones", [1, NUM_CLASS], f32r, kind="ExternalInput").ap()
    sbias = nc.dram_tensor("sbias", [NUM_CLASS, 1], f32, kind="ExternalInput").ap()
    outT = nc.dram_tensor("outT", [NUM_CLASS, ROWS], f32, kind="ExternalOutput").ap()

    with tile.TileContext(nc) as tc, ExitStack() as ctx:
        pconst = ctx.enter_context(tc.tile_pool(name="pconst", bufs=1))
        pdata = ctx.enter_context(tc.tile_pool(name="pdata", bufs=8))
        psq = ctx.enter_context(tc.tile_pool(name="psq", bufs=5))
        pep = ctx.enter_context(tc.tile_pool(name="pep", bufs=1))
        pev = ctx.enter_context(tc.tile_pool(name="pev", bufs=2))
        pz = ctx.enter_context(tc.tile_pool(name="pz", bufs=2, space="PSUM"))
        ps = ctx.enter_context(tc.tile_pool(name="ps", bufs=2, space="PSUM"))
        pr = ctx.enter_context(tc.tile_pool(name="pr", bufs=4, space="PSUM"))

        # all consts ride SWDGE so the sync HWDGE ring starts the input
        # stream immediately (the stream is the conveyor; the warmup can wait)
        wt_s = pconst.tile([128, N_ETILES * NUM_CLASS], bf16)
        nc.gpsimd.dma_start(out=wt_s, in_=wt)
        ones_s = pconst.tile([128, 1], bf16)
        nc.gpsimd.dma_start(out=ones_s, in_=onesv)
        selz_s = pconst.tile([128, NUM_CLASS], f32r)
        nc.gpsimd.dma_start(out=selz_s, in_=selz)
        selss_s = pconst.tile([128, 1], bf16)
        nc.gpsimd.dma_start(out=selss_s, in_=selss)
        # sones placed on partition 32 (same partition as the reduced sumsq)
        sones_s = pconst.tile([128, NUM_CLASS], f32r)
        nc.gpsimd.dma_start(out=sones_s[32:33, :], in_=sones)
        sbias_s = pconst.tile([NUM_CLASS, 1], f32)
        nc.gpsimd.dma_start(out=sbias_s, in_=sbias)
        zero128_s = pconst.tile([128, 1], f32)
        nc.vector.memset(zero128_s, 0.0)
        # rsqrt via exp(-0.5*ln(ss) + ln(S)): folds the *S scale in for free
        lnS_s = pconst.tile([128, 1], f32)
        nc.vector.memset(lnS_s, float(np.log(S)))

        lnss_s = pep.tile([128, ROWS], f32)
        rnorm_s = pep.tile([128, ROWS], f32r)
        zr_s = pep.tile([NUM_CLASS, ROWS], f32)
        out_s = pep.tile([NUM_CLASS, ROWS], f32)

        # per-quarter accumulators, one psum bank each: tile t's z partial at
        # partitions 32t..32t+3, its sumsq partial at partition 32t
        zt_ps = [None] * N_Q
        ss_ps = [None] * N_Q
        xts = [[None] * N_H for _ in range(N_Q)]
        x2s = [[None] * N_H for _ in range(N_Q)]

        def issue_dma(j):
            # all inputs on the SP HWDGE ring (the ACT ring stalls the ACT
            # queue); every xt tile is pool-resident so no dma_start ever
            # waits on buffer reuse.  Mixed granularity: quarter 0 and the
            # final half land in 2-panel pieces (early start / tail chasing),
            # middle quarters in single 2 MiB transfers (peak HBM rate).
            for h in range(N_H):
                xts[j][h] = pdata.tile([128, 8, CW], bf16, tag="xt", name=f"xt{j}{h}")
            if j == 0:
                for h in range(N_H):
                    for i in range(0, 8, 2):
                        nc.sync.dma_start(
                            out=xts[j][h][:, i : i + 2, :],
                            in_=feaT[j, h, :, i : i + 2, :],
                        )
            else:
                for h in range(N_H):
                    nc.sync.dma_start(out=xts[j][h], in_=feaT[j, h])

        def square(j, h):
            x2s[j][h] = psq.tile([128, 8, CW], bf16, tag="x2", name=f"x2{j}{h}")
            if (j, h) in ACT_SQ:
                nc.scalar.activation(
                    out=x2s[j][h],
                    in_=xts[j][h],
                    func=mybir.ActivationFunctionType.Square,
                    bias=zero128_s,
                    scale=1.0,
                )
            elif (j, h) in GPS_SQ:
                nc.gpsimd.tensor_mul(x2s[j][h], xts[j][h], xts[j][h])
            elif j == N_Q - 1 and h == N_H - 1:
                # the very last half: fine-grained pair squares so the tail
                # sumsq matmuls chase pairs instead of one 4096-wide op
                for i in range(0, 8, 2):
                    nc.vector.tensor_mul(
                        x2s[j][h][:, i : i + 2, :],
                        xts[j][h][:, i : i + 2, :],
                        xts[j][h][:, i : i + 2, :],
                    )
            else:
                nc.vector.tensor_mul(x2s[j][h], xts[j][h], xts[j][h])

        def z_mms(j, h, start, stop):
            for i in range(8):
                p = 8 * h + i
                t = p % 4
                nc.tensor.matmul(
                    zt_ps[j][32 * t : 32 * t + 4, :],
                    wt_s[:, p * NUM_CLASS : (p + 1) * NUM_CLASS],
                    xts[j][h][:, i, :],
                    start=start and i < 4,
                    stop=stop and i >= 4,
                    tile_position=(0, 32 * t),
                )

        def ss_mms(j, h, start, stop):
            for i in range(8):
                t = (8 * h + i) % 4
                nc.tensor.matmul(
                    ss_ps[j][32 * t : 32 * t + 1, :],
                    ones_s,
                    x2s[j][h][:, i, :],
                    start=start and i < 4,
                    stop=stop and i >= 4,
                    tile_position=(0, 32 * t),
                )

        def epi_part1(j):
            # evacuate the two accumulator banks, reduce, ln
            bsl = slice(j * CW, (j + 1) * CW)
            zsb = pev.tile([128, CW], f32r, tag="zsb", name=f"zsb{j}")
            nc.vector.tensor_copy(zsb, zt_ps[j])
            ssb = pev.tile([128, CW], bf16, tag="ssb", name=f"ssb{j}")
            nc.scalar.copy(ssb, ss_ps[j])
            ra = pr.tile([128, CW], f32, tag="rx", name=f"ra{j}")
            nc.tensor.matmul(
                ra[0:NUM_CLASS, :],
                selz_s,
                zsb,
                start=True,
                stop=True,
                tile_position=(0, 0),
            )
            nc.tensor.matmul(
                ra[32:33, :],
                selss_s,
                ssb,
                start=True,
                stop=True,
                tile_position=(0, 32),
            )
            nc.scalar.activation(
                out=lnss_s[32:33, bsl],
                in_=ra[32:33, :],
                func=mybir.ActivationFunctionType.Ln,
                bias=zero128_s[32:33],
                scale=1.0,
            )
            nc.scalar.activation(
                out=rnorm_s[32:33, bsl],
                in_=lnss_s[32:33, bsl],
                func=mybir.ActivationFunctionType.Exp,
                bias=lnS_s[32:33],
                scale=-0.5,
            )
            return ra

        def epi_part2(j, ra):
            # rnorm broadcast (k=1 matmul on row tile 32), scale, bias, out
            bsl = slice(j * CW, (j + 1) * CW)
            rb = pr.tile([128, CW], f32, tag="rx", name=f"rb{j}")
            nc.tensor.matmul(
                rb[0:NUM_CLASS, :],
                sones_s[32:33, :],
                rnorm_s[32:33, bsl],
                start=True,
                stop=True,
                tile_position=(32, 0),
            )
            # DVE can read only one PSUM operand per op: stage rnb in SBUF
            rnbs = pev.tile([NUM_CLASS, CW], f32, tag="rnbs", name=f"rnbs{j}")
            nc.vector.tensor_copy(rnbs, rb[0:NUM_CLASS, :])
            nc.vector.tensor_mul(zr_s[:, bsl], ra[0:NUM_CLASS, :], rnbs)
            nc.vector.tensor_scalar_add(
                out_s[:, bsl], in0=zr_s[:, bsl], scalar1=sbias_s
            )
            nc.sync.dma_start(out=outT[:, bsl], in_=out_s[:, bsl])

        issue_dma(0)
        # activation-table preload (Square+Ln+Exp in one set) while the ACT
        # queue is otherwise empty; quarter 0's squares avoid ACT
        nlx_id = list(get_activation_tables(nc.m.arch)).index(
            "natural_log_exp_and_others"
        )
        nc.scalar.add_instruction(
            mybir.InstLoadActFuncSet(name=f"I-{nc.next_id()}", act_func_set_id=nlx_id)
        )

        ras = [None] * N_Q
        for j in range(N_Q):
            zt_ps[j] = pz.tile([128, CW], f32, tag="zt", name=f"zt{j}")
            ss_ps[j] = ps.tile([128, CW], f32, tag="ss", name=f"ss{j}")
            if j == 0:
                # pre-warm the PE while the first transfer is in flight: the
                # first real z matmul's start=True resets the garbage
                for w in range(24):
                    t = w % 4
                    nc.tensor.matmul(
                        zt_ps[0][32 * t : 32 * t + 4, 0:64],
                        wt_s[:, 0:NUM_CLASS],
                        wt_s[:, 0:64],
                        start=True,
                        stop=True,
                        tile_position=(0, 32 * t),
                    )
            if j + 1 < N_Q:
                issue_dma(j + 1)
            for h in range(N_H):
                square(j, h)
                z_mms(j, h, start=(h == 0), stop=(h == N_H - 1))
                if h == 0 and j > 0:
                    # previous quarter's second-half sumsq: its squares are a
                    # full quarter old, so the in-order PE queue never stalls
                    ss_mms(j - 1, N_H - 1, start=False, stop=True)
                    if j > 1:
                        # part2's broadcast matmul waits on the Ln/Exp chain;
                        # emitted a further half-slot later so it never sits
                        # in the PE queue ahead of stream/sumsq matmuls
                        epi_part2(j - 2, ras[j - 2])
                if h == 1:
                    ss_mms(j, 0, start=True, stop=False)
                    if j > 0:
                        ras[j - 1] = epi_part1(j - 1)
        ss_mms(N_Q - 1, N_H - 1, start=False, stop=True)
        epi_part2(N_Q - 2, ras[N_Q - 2])
        ras[N_Q - 1] = epi_part1(N_Q - 1)
        epi_part2(N_Q - 1, ras[N_Q - 1])

    nc.compile()
    return nc


def _get_nc():
    if "nc" not in _CACHE:
        _CACHE["nc"] = _build_nc()
    return _CACHE["nc"]


def _stage_inputs(fea, W, b):
    import ml_dtypes

    fea = np.asarray(fea, dtype=np.float32)
    W = np.asarray(W, dtype=np.float32)
    b = np.asarray(b, dtype=np.float32)

    # wt[p, 4t+c] = W[c, 128t+p]
    wt = np.ascontiguousarray(
        W.reshape(NUM_CLASS, N_ETILES, 128).transpose(2, 1, 0).reshape(128, -1)
    ).astype(ml_dtypes.bfloat16)
    onesv = np.ones((128, 1), dtype=ml_dtypes.bfloat16)
    selz = np.zeros((128, NUM_CLASS), dtype=np.float32)
    selss = np.zeros((128, 1), dtype=ml_dtypes.bfloat16)
    for t in range(4):
        for c in range(NUM_CLASS):
            selz[32 * t + c, c] = 1.0
        selss[32 * t, 0] = 1.0
    # the *S scale is folded into the exp(-0.5*ln(ss)+ln(S)) rsqrt, so the
    # class-broadcast matmul uses plain ones
    sones = np.ones((1, NUM_CLASS), dtype=np.float32)
    sbias = (S * b).reshape(NUM_CLASS, 1).astype(np.float32)

    in_maps = []
    for i in range(N_CORES):
        shard = fea[i * ROWS : (i + 1) * ROWS, :]
        feaT = np.ascontiguousarray(shard.T)  # [EMB, ROWS]
        # [quarter j, half h, partition p, panel a, col b]
        fea5 = np.ascontiguousarray(
            feaT.reshape(N_H, 8, 128, N_Q, CW).transpose(3, 0, 2, 1, 4)
        ).astype(ml_dtypes.bfloat16)
        in_maps.append(
            {
                "feaT": fea5,
                "wt": wt,
                "onesv": onesv,
                "selz": selz,
                "selss": selss,
                "sones": sones,
                "sbias": sbias,
            }
        )
    return in_maps


def run(fea, W, b, trace=False):
    from concourse.bass_utils import run_bass_kernel_spmd

    nc = _get_nc()
    in_maps = _stage_inputs(fea, W, b)
    res = run_bass_kernel_spmd(nc, in_maps, core_ids=list(range(N_CORES)), trace=trace)
    out = np.empty((BATCH, NUM_CLASS), dtype=np.float32)
    for i in range(N_CORES):
        out[i * ROWS : (i + 1) * ROWS, :] = res.results[i]["outT"].T
    return out, res


def kernel(fea, W, b):
    out, _ = run(fea, W, b, trace=False)
    return out
